# revision 1
# baseline (speedup 1.0000x reference)
"""Trainium2 Bass kernel for nn_Attention_9096740733536 (sparse_attention).

Sharding: data-parallel over the QB (task) dim across 8 cores (2 tasks/core),
one mid-kernel AllReduce of [feat_corr partials | q_global | k_global] sums.
The attention math is algebraically collapsed: mixed scores are linear (no
softmax), so
  out[h,q] = alpha_h*(Fq/qn) @ ((Fk/kn)^T @ Fv) + ww_h*q_ratio (x) (k_ratio^T Fv)
with 128x128 inner matrices instead of 512x512 score matrices, and layernorm
is folded into the input projection via rank-1 PSUM augmentation.
"""
import numpy as np
from contextlib import ExitStack

import concourse.bass as bass
import concourse.tile as tile
from concourse import bacc, mybir
from concourse import bass_utils
from concourse._compat import with_exitstack

F32 = mybir.dt.float32
F32R = mybir.dt.float32r
AF = mybir.ActivationFunctionType
ALU = mybir.AluOpType
AX = mybir.AxisListType

H, D, DIM = 8, 128, 1024
QB, N = 16, 512
N_CORES = 8
T = QB * N // N_CORES          # 1024 tokens per core
NT = T // 128                  # 8 token tiles per core
NTASK = T // N                 # 2 tasks per core
LN_EPS = 1e-5
TOK_ALL = float(QB * N)


@with_exitstack
def attn_kernel(ctx: ExitStack, tc: tile.TileContext, outs, ins, n_cores=N_CORES):
    nc = tc.nc
    y = outs[0]
    (xn_q, xn_k, xn_v, xT_q, xT_k, xT_v, Wp_d, WoT_d, negu_d, vrow_d,
     bout_d, ones_d, ident_d, mask_d, wp1T_d, wp2T_d, b1_d, gbc_d, bbc_d,
     b2bc_d) = ins

    consts = ctx.enter_context(tc.tile_pool(name="consts", bufs=1))
    fpool = ctx.enter_context(tc.tile_pool(name="fpool", bufs=1))
    stat1 = ctx.enter_context(tc.tile_pool(name="stat1", bufs=1))
    dram = ctx.enter_context(tc.tile_pool(name="dram", bufs=1, space="DRAM"))

    ps_proj = ctx.enter_context(tc.tile_pool(name="ps_proj", bufs=3, space="PSUM"))
    ps_fc = ctx.enter_context(tc.tile_pool(name="ps_fc", bufs=2, space="PSUM"))
    ps_gk = ctx.enter_context(tc.tile_pool(name="ps_gk", bufs=1, space="PSUM"))
    ps_o1 = ctx.enter_context(tc.tile_pool(name="ps_o1", bufs=1, space="PSUM"))
    ps_small = ctx.enter_context(tc.tile_pool(name="ps_small", bufs=1, space="PSUM"))

    # ---- small constants (long-lived) ----
    ident = consts.tile([128, 128], F32)
    nc.sync.dma_start(ident[:], ident_d[:])
    bout = consts.tile([1, DIM], F32R)
    nc.sync.dma_start(bout[:], bout_d[:].bitcast(F32R))
    onesr = consts.tile([1, 128], F32R)
    nc.sync.dma_start(onesr[:], ones_d[0:1, :].bitcast(F32R))
    ones = consts.tile([128, 8], F32)
    nc.sync.dma_start(ones[:], ones_d[:, 0:8])
    mask_nd = consts.tile([128, H * 128], F32)
    nc.scalar.dma_start(mask_nd[:], mask_d[:])
    wp1T = consts.tile([128, 256], F32)
    nc.scalar.dma_start(wp1T[:], wp1T_d[:])
    wp2T = consts.tile([128, 3], F32)
    nc.scalar.dma_start(wp2T[:], wp2T_d[:])
    b1row = consts.tile([1, 128], F32)
    nc.scalar.dma_start(b1row[:], b1_d[:])
    ones8 = consts.tile([1, 8], F32)
    nc.sync.dma_start(ones8[:], ones_d[0:1, 0:8])
    gbc = consts.tile([8, 128], F32)
    nc.scalar.dma_start(gbc[:], gbc_d[:])
    bbc = consts.tile([8, 128], F32)
    nc.scalar.dma_start(bbc[:], bbc_d[:])
    b2bc = consts.tile([8, 3], F32)
    nc.scalar.dma_start(b2bc[:], b2bc_d[:])
    eps = consts.tile([128, 1], F32)
    nc.vector.memset(eps[:], LN_EPS)

    # ---- persistent F tensors: [128 tok, t*1024 + h*128 + d] ----
    Fq = fpool.tile([128, NT * DIM], F32)
    Fk = fpool.tile([128, NT * DIM], F32)
    Fv = fpool.tile([128, NT * DIM], F32)
    sq_scr = stat1.tile([128, DIM], F32)     # ACT square scratch (write-only)

    xns = [xn_q, xn_k, xn_v]
    xTs = [xT_q, xT_k, xT_v]
    Fs = [Fq, Fk, Fv]

    # ======== Phase 1: folded-LN projection (scoped pools) ========
    with tc.tile_pool(name="ph1", bufs=1) as ph1, \
         tc.tile_pool(name="xpool", bufs=3) as xpool, \
         tc.tile_pool(name="spool", bufs=3) as spool:
        Wp = ph1.tile([128, 8 * DIM], F32R)
        for s in range(8):
            nc.gpsimd.dma_start(Wp[:, s * DIM:(s + 1) * DIM],
                                Wp_d[:, s * DIM:(s + 1) * DIM].bitcast(F32R))
        negu = ph1.tile([1, DIM], F32R)
        nc.sync.dma_start(negu[:], negu_d[:].bitcast(F32R))
        vrow = ph1.tile([1, DIM], F32R)
        nc.sync.dma_start(vrow[:], vrow_d[:].bitcast(F32R))
        for t in range(NT):
            st = spool.tile([128, 12], F32, tag="st")
            bn6 = spool.tile([128, 36], F32, tag="bn6")
            rsig = spool.tile([128, 3], F32, tag="rsig")
            for i in range(3):
                xn = xpool.tile([128, DIM], F32, tag="xn")
                nc.sync.dma_start(xn[:], xns[i][t * 128:(t + 1) * 128, :])
                nc.vector.bn_stats(bn6[:, i * 12:i * 12 + 6], xn[:, 0:512])
                nc.vector.bn_stats(bn6[:, i * 12 + 6:i * 12 + 12],
                                   xn[:, 512:1024])
                # (mean, var) pair -> st cols (6+i, 9+i via sqrt)
                nc.vector.bn_aggr(st[:, 2 * i:2 * i + 2],
                                  bn6[:, i * 12:i * 12 + 12])
            # st cols 0,2,4 = mu ; 1,3,5 = var
            nc.vector.tensor_copy(st[:, 6:9], st[:, 0:6:2])
            nc.scalar.activation(st[:, 9:12], st[:, 1:6:2], AF.Sqrt,
                                 bias=eps[:])
            nc.vector.reciprocal(rsig[:], st[:, 9:12])
            # transpose [mu|sig] (cols 6..11) -> rows [6, 128] -> flat [1, 768]
            trp = ps_small.tile([6, 128], F32, tag="sm")
            nc.tensor.transpose(trp[:], st[:, 6:12], ident[:])
            rows6 = spool.tile([6, 128], F32R, tag="rows6")
            nc.scalar.copy(rows6[:], trp[:])
            rows = spool.tile([1, 768], F32R, tag="rows")
            nc.scalar.dma_start(rows[:], rows6[:])
            for i in range(3):
                xT_t = xpool.tile([128, DIM], F32R, tag="xT")
                nc.sync.dma_start(xT_t[:],
                                  xTs[i][:, t * DIM:(t + 1) * DIM].bitcast(F32R))
                for half in range(2):
                    o = half * 512
                    acc = ps_proj.tile([128, 512], F32, tag="proj")
                    for s in range(8):
                        nc.tensor.matmul(
                            acc[:], xT_t[:, s * 128:(s + 1) * 128],
                            Wp[:, s * DIM + o: s * DIM + o + 512],
                            start=(s == 0), stop=False)
                    nc.tensor.matmul(acc[:], rows[:, i * 128:(i + 1) * 128],
                                     negu[:, o:o + 512], start=False, stop=False)
                    nc.tensor.matmul(acc[:], rows[:, (3 + i) * 128:(4 + i) * 128],
                                     vrow[:, o:o + 512], start=False, stop=True)
                    dst = Fs[i][:, t * DIM + o: t * DIM + o + 512]
                    if (i + half) % 2 == 0:
                        nc.scalar.mul(dst, acc[:], rsig[:, i:i + 1])
                    else:
                        nc.vector.tensor_scalar_mul(dst, acc[:],
                                                    rsig[:, i:i + 1])

    # ======== Phase 2: F stats, feat_corr partials, q/k globals ========
    late = ctx.enter_context(tc.tile_pool(name="late", bufs=1))
    WoT = late.tile([128, 8 * DIM], F32R)
    nc.gpsimd.dma_start(WoT[:], WoT_d[:].bitcast(F32R))

    qss = stat1.tile([128, 64], F32)   # col t*8+h : sumsq over d of Fq
    qsm = stat1.tile([128, 64], F32)   # sums over d
    kss = stat1.tile([128, 64], F32)
    ksm = stat1.tile([128, 64], F32)
    qmean = stat1.tile([128, 64], F32)
    qninv = stat1.tile([128, 64], F32)
    kninv = stat1.tile([128, 64], F32)
    kn = stat1.tile([128, 64], F32)
    qr = stat1.tile([128, 64], F32)
    kr = stat1.tile([128, 64], F32)
    rscr = stat1.tile([128, 96], F32)  # ratio-chain scratch (3x32 per half)

    def derived(ss, sm, ninv, ratio, s, n_out=None):
        # ninv = 1/sqrt(ss); var = ss/127 - sm^2/(128*127)
        # ratio = 2*min(var,1)/(var+1)
        w = s.stop - s.start
        if n_out is not None:
            nc.scalar.activation(n_out[:, s], ss[:, s], AF.Sqrt)
            nc.vector.reciprocal(ninv[:, s], n_out[:, s])
        else:
            nc.scalar.activation(ninv[:, s], ss[:, s], AF.Sqrt)
            nc.vector.reciprocal(ninv[:, s], ninv[:, s])
        t1 = rscr[:, 0:w]
        nc.vector.tensor_tensor(t1, sm[:, s], sm[:, s], op=ALU.mult)
        nc.vector.tensor_scalar_mul(t1, t1, 1.0 / (D * (D - 1)))
        t2 = rscr[:, w:2 * w]
        nc.vector.tensor_scalar_mul(t2, ss[:, s], 1.0 / (D - 1))
        var = rscr[:, 2 * w:3 * w]
        nc.vector.tensor_tensor(var, t2, t1, op=ALU.subtract)
        nc.vector.tensor_scalar(t1, var, 1.0, 2.0, ALU.min, ALU.mult)
        nc.vector.tensor_scalar_add(t2, var, 1.0)
        nc.vector.reciprocal(t2, t2)
        nc.vector.tensor_tensor(ratio[:, s], t1, t2, op=ALU.mult)

    for jh in range(NTASK):
        for t in range(4 * jh, 4 * jh + 4):
            nc.vector.reduce_sum(
                qsm[:, t * 8:(t + 1) * 8],
                Fq[:, t * DIM:(t + 1) * DIM].rearrange("p (h d) -> p h d", h=8),
                axis=AX.X)
            nc.vector.reduce_sum(
                ksm[:, t * 8:(t + 1) * 8],
                Fk[:, t * DIM:(t + 1) * DIM].rearrange("p (h d) -> p h d", h=8),
                axis=AX.X)
            for h in range(H):
                sl = slice(t * DIM + h * 128, t * DIM + h * 128 + 128)
                nc.scalar.activation(sq_scr[:, 0:128], Fq[:, sl], AF.Square,
                                     accum_out=qss[:, t * 8 + h:t * 8 + h + 1])
                nc.scalar.activation(sq_scr[:, 128:256], Fk[:, sl], AF.Square,
                                     accum_out=kss[:, t * 8 + h:t * 8 + h + 1])
        s = slice(jh * 32, jh * 32 + 32)
        # NOTE: qmean holds NEGATED means (used as ACT bias for centering)
        nc.vector.tensor_scalar_mul(qmean[:, s], qsm[:, s], -1.0 / D)
        derived(qss, qsm, qninv, qr, s)
        derived(kss, ksm, kninv, kr, s, n_out=kn)
        # absorb kn into k_ratio: mv uses scaled Fv, so kr must carry kn back
        nc.vector.tensor_tensor(kr[:, s], kr[:, s], kn[:, s], op=ALU.mult)
        # scale Fv in place by 1/kn (only consumer is the M/mv stage)
        for t in range(4 * jh, 4 * jh + 4):
            for h in range(H):
                sl = slice(t * DIM + h * 128, t * DIM + h * 128 + 128)
                nc.vector.tensor_scalar(Fv[:, sl], Fv[:, sl],
                                        kninv[:, t * 8 + h:t * 8 + h + 1],
                                        None, ALU.mult)

    # ======== Phase 4a: allreduce-independent M/mv stage ========
    # M = Fk^T @ (Fv/kn) and mv = (kr*kn)^T @ (Fv/kn) per (head, task),
    # evicted UNSCALED (alpha/ww applied post-allreduce). Placed BEFORE the
    # feat_corr stage so the in-order PE stream overlaps the phase-1 tail.
    attn = ctx.enter_context(tc.tile_pool(name="attn", bufs=1))
    mm_raw = {}
    mv_raw = {}
    for j in range(NTASK):
        for h in range(H):
            mm_ps = ps_fc.tile([128, 128], F32, tag="fc128", name="mm_ps")
            mv_ps = ps_small.tile([1, 128], F32, tag="sm", name="mv_ps")
            for ti in range(4):
                t = 4 * j + ti
                sl = slice(t * DIM + h * 128, t * DIM + h * 128 + 128)
                nc.tensor.matmul(mm_ps[:], Fk[:, sl], Fv[:, sl],
                                 start=(ti == 0), stop=(ti == 3))
                nc.tensor.matmul(mv_ps[:], kr[:, t * 8 + h:t * 8 + h + 1],
                                 Fv[:, sl], start=(ti == 0), stop=(ti == 3))
            mm = attn.tile([128, 128], F32R, tag=f"mm{h}{j}", name="mm")
            nc.scalar.copy(mm[:], mm_ps[:])
            mv = attn.tile([1, 128], F32R, tag=f"mv{h}{j}", name="mv")
            nc.scalar.copy(mv[:], mv_ps[:])
            mm_raw[(h, j)] = mm
            mv_raw[(h, j)] = mv

    # feat_corr partials (per head) + q/k global sums (single PSUM group)
    # t-outer emission so no engine stream blocks on the last proj tile.
    ar_in = dram.tile([128, H * 128 + 16], F32)
    ar_out = dram.tile([128, H * 128 + 16], F32)
    gk_ps = ps_gk.tile([128, 16], F32, tag="gk")
    with tc.tile_pool(name="ph2", bufs=2) as ph2, \
         tc.tile_pool(name="qcpool", bufs=64) as qcpool:
        qc_tiles = {}
        for t in range(NT):
            for h in range(H):
                sl = slice(t * DIM + h * 128, t * DIM + h * 128 + 128)
                qc = qcpool.tile([128, 128], mybir.dt.bfloat16, tag="qc",
                                 name="qc")
                nc.scalar.activation(qc[:], Fq[:, sl], AF.Identity,
                                     bias=qmean[:, t * 8 + h:t * 8 + h + 1])
                qc_tiles[(t, h)] = qc
                first = (h == 0 and t == 0)
                last = (h == H - 1 and t == NT - 1)
                nc.tensor.matmul(gk_ps[:, h:h + 1], Fq[:, sl], ones[:, 0:1],
                                 start=first, stop=last, skip_group_check=True)
                nc.tensor.matmul(gk_ps[:, 8 + h:9 + h], Fk[:, sl], ones[:, 0:1],
                                 start=False, stop=False, skip_group_check=True)
        for h in range(H):
            fc_ps = ps_fc.tile([128, 128], F32, tag="fc128", name="fc_ps")
            for t in range(NT):
                nc.tensor.matmul(fc_ps[:], qc_tiles[(t, h)][:],
                                 qc_tiles[(t, h)][:],
                                 start=(t == 0), stop=(t == NT - 1))
            fc_sb = ph2.tile([128, 128], F32, tag="fcsb", name="fc_sb")
            nc.vector.tensor_copy(fc_sb[:], fc_ps[:])
            nc.sync.dma_start(ar_in[:, h * 128:(h + 1) * 128], fc_sb[:])
        gk_sb = ph2.tile([128, 16], F32, tag="gksb", name="gk_sb")
        nc.scalar.copy(gk_sb[:], gk_ps[:])
        nc.sync.dma_start(ar_in[:, H * 128:H * 128 + 16], gk_sb[:])

    # in-place Fq <- Fq/qn (after feat_corr reads; gates only phase 4b)
    for h in range(H):
        for t in range(NT):
            sl = slice(t * DIM + h * 128, t * DIM + h * 128 + 128)
            c = slice(t * 8 + h, t * 8 + h + 1)
            nc.vector.tensor_scalar(Fq[:, sl], Fq[:, sl], qninv[:, c], None,
                                    ALU.mult)

    # ======== AllReduce ========
    if n_cores > 1:
        nc.gpsimd.collective_compute(
            "AllReduce", ALU.add,
            replica_groups=[list(range(n_cores))],
            ins=[ar_in.opt()], outs=[ar_out.opt()])
    else:  # single-core sim variant: allreduce over one core == copy
        nc.sync.dma_start(ar_out[:], ar_in[:])
    ar = late.tile([128, H * 128 + 16], F32)
    nc.sync.dma_start(ar[:], ar_out[:])
    arg = ar[:, H * 128:H * 128 + 16]

    # ======== Phase 3: decorr scale + weight predictor ========
    ssq = stat1.tile([128, 8], F32)
    msk = late.tile([128, H * 128], F32)
    nc.vector.tensor_tensor(msk[:], ar[:, 0:H * 128], mask_nd[:], op=ALU.mult)
    nc.scalar.activation(sq_scr[:, 0:H * 128], msk[:], AF.Square,
                         scale=1.0 / TOK_ALL)
    nc.vector.reduce_sum(ssq[:],
                         sq_scr[:, 0:H * 128].rearrange("p (h d) -> p h d", h=8),
                         axis=AX.X)
    ss_ps = ps_small.tile([8, 8], F32, tag="sm", name="ss_ps")
    nc.tensor.matmul(ss_ps[:], ssq[:], ones[:, 0:8], start=True, stop=True)
    dsc = stat1.tile([8, 8], F32)
    nc.scalar.activation(dsc[:, 0:1], ss_ps[0:8, 0:1], AF.Sqrt)
    nc.scalar.activation(dsc[:, 1:2], dsc[:, 0:1], AF.Exp, scale=-5.0 / (D * D))

    featsq = stat1.tile([128, 8], F32)
    nc.vector.tensor_scalar_mul(featsq[:], arg[:, 0:8], 1.0 / TOK_ALL)
    featsk = stat1.tile([128, 8], F32)
    nc.vector.tensor_scalar_mul(featsk[:], arg[:, 8:16], 1.0 / TOK_ALL)
    h1_ps = ps_small.tile([8, 128], F32, tag="sm", name="h1_ps")
    nc.tensor.matmul(h1_ps[:], featsq[:], wp1T[:, 0:128], start=True, stop=False)
    nc.tensor.matmul(h1_ps[:], featsk[:], wp1T[:, 128:256], start=False,
                     stop=False)
    nc.tensor.matmul(h1_ps[:], ones8[:], b1row[:], start=False, stop=True)
    h1 = stat1.tile([8, 128], F32)
    nc.scalar.copy(h1[:], h1_ps[:])
    w_mu = stat1.tile([8, 4], F32)
    nc.vector.reduce_sum(w_mu[:, 0:1], h1[:], axis=AX.X)
    nc.vector.tensor_scalar_mul(w_mu[:, 0:1], w_mu[:, 0:1], 1.0 / D)
    nc.scalar.activation(sq_scr[0:8, 0:128], h1[:], AF.Square,
                         accum_out=w_mu[:, 1:2])
    nc.vector.tensor_scalar_mul(w_mu[:, 1:2], w_mu[:, 1:2], 1.0 / D)
    nc.vector.tensor_tensor(w_mu[:, 2:3], w_mu[:, 0:1], w_mu[:, 0:1], op=ALU.mult)
    nc.vector.tensor_tensor(w_mu[:, 2:3], w_mu[:, 1:2], w_mu[:, 2:3],
                            op=ALU.subtract)
    nc.scalar.activation(w_mu[:, 3:4], w_mu[:, 2:3], AF.Sqrt, bias=eps[0:8, :])
    nc.vector.reciprocal(w_mu[:, 3:4], w_mu[:, 3:4])
    h1n = stat1.tile([8, 128], F32)
    nc.vector.tensor_scalar(h1n[:], h1[:], w_mu[:, 0:1], w_mu[:, 3:4],
                            ALU.subtract, ALU.mult)
    nc.vector.tensor_tensor(h1n[:], h1n[:], gbc[:], op=ALU.mult)
    nc.vector.tensor_tensor(h1n[:], h1n[:], bbc[:], op=ALU.add)
    nc.vector.tensor_scalar_max(h1n[:], h1n[:], 0.0)
    h1T_ps = ps_small.tile([128, 8], F32, tag="sm", name="h1T_ps")
    nc.tensor.transpose(h1T_ps[:], h1n[:], ident[0:8, 0:8])
    h1T = stat1.tile([128, 8], F32)
    nc.scalar.copy(h1T[:], h1T_ps[:])
    lg_ps = ps_small.tile([8, 3], F32, tag="sm", name="lg_ps")
    nc.tensor.matmul(lg_ps[:], h1T[:], wp2T[:], start=True, stop=True)
    lg = stat1.tile([8, 8], F32)
    nc.scalar.copy(lg[:, 0:3], lg_ps[:])
    nc.vector.tensor_tensor(lg[:, 0:3], lg[:, 0:3], b2bc[:], op=ALU.add)
    # logits are O(1): skip the (mathematically redundant) max-subtraction
    nc.scalar.activation(lg[:, 0:3], lg[:, 0:3], AF.Exp)
    nc.vector.reduce_sum(lg[:, 4:5], lg[:, 0:3], axis=AX.X)
    nc.vector.reciprocal(lg[:, 4:5], lg[:, 4:5])
    nc.vector.tensor_scalar(lg[:, 0:3], lg[:, 0:3], lg[:, 4:5], None, ALU.mult)
    # alpha = w0 + w1*dsc ; ww = w2 ; broadcast to 128 partitions
    aw = stat1.tile([8, 2], F32)
    nc.vector.tensor_tensor(aw[:, 0:1], lg[:, 1:2], dsc[:, 1:2], op=ALU.mult)
    nc.vector.tensor_tensor(aw[:, 0:1], aw[:, 0:1], lg[:, 0:1], op=ALU.add)
    nc.vector.tensor_copy(aw[:, 1:2], lg[:, 2:3])
    awT_ps = ps_small.tile([2, 8], F32, tag="sm", name="awT_ps")
    nc.tensor.transpose(awT_ps[:], aw[:], ident[0:8, 0:8])
    awT = stat1.tile([2, 8], F32)
    nc.scalar.copy(awT[:], awT_ps[:])
    aw_flat = stat1.tile([1, 16], F32)
    nc.scalar.dma_start(aw_flat[:], awT[:])
    abc = stat1.tile([128, 8], F32)
    nc.gpsimd.partition_broadcast(abc[:], aw_flat[:, 0:8])
    wbc = stat1.tile([128, 8], F32)
    nc.gpsimd.partition_broadcast(wbc[:], aw_flat[:, 8:16])

    # ======== Phase 4b + 5: scaled attention + output projection ========
    with tc.tile_pool(name="ph4", bufs=2) as ph4, \
         tc.tile_pool(name="o1pool", bufs=10) as o1pool:
        o1_tiles = {}
        for j in range(NTASK):
            for h in range(H):
                mm_sb = ph4.tile([128, 128], F32R, tag="mmsb", name="mm_sb")
                nc.vector.tensor_scalar(mm_sb[:], mm_raw[(h, j)][:],
                                        abc[:, h:h + 1], None, ALU.mult)
                mv_sb = ph4.tile([1, 128], F32R, tag="mvsb", name="mv_sb")
                nc.vector.tensor_scalar(mv_sb[:], mv_raw[(h, j)][:],
                                        wbc[0:1, h:h + 1], None, ALU.mult)

                # q_ratio row for this (h, j): [1, 512]
                c0 = 4 * j * 8 + h
                wq_ps = ps_small.tile([4, 128], F32, tag="sm", name="wq_ps")
                nc.tensor.transpose(wq_ps[:], qr[:, c0:c0 + 25:8], ident[:])
                wq4 = ph4.tile([4, 128], F32R, tag="wq4", name="wq4")
                nc.scalar.copy(wq4[:], wq_ps[:])
                wqr = ph4.tile([1, 512], F32R, tag="wqr", name="wqr")
                nc.scalar.dma_start(wqr[:], wq4[:])

                fqTs = ph4.tile([128, 512], F32R, tag="fqTs", name="fqTs")
                for ti in range(4):
                    t = 4 * j + ti
                    sl = slice(t * DIM + h * 128, t * DIM + h * 128 + 128)
                    qsT_ps = ps_fc.tile([128, 128], F32, tag="fc128",
                                        name="qsT_ps")
                    nc.tensor.transpose(qsT_ps[:], Fq[:, sl], ident[:])
                    nc.scalar.copy(fqTs[:, ti * 128:(ti + 1) * 128], qsT_ps[:])

                o1_ps = ps_o1.tile([128, 512], F32, tag="o1", name="o1_ps")
                nc.tensor.matmul(o1_ps[:], mm_sb[:], fqTs[:], start=True,
                                 stop=False)
                nc.tensor.matmul(o1_ps[:], mv_sb[:], wqr[:],
                                 start=False, stop=True)
                o1 = o1pool.tile([128, 512], F32R, tag="o1sb", name="o1_sb")
                nc.vector.tensor_copy(o1[:], o1_ps[:])
                o1_tiles[(h, j)] = o1

            # ---- output projection for this task ----
            for t in range(4 * j, 4 * j + 4):
                ti = t % 4
                for half in range(2):
                    o = half * 512
                    op_ps = ps_proj.tile([128, 512], F32, tag="proj",
                                         name="op_ps")
                    for h in range(H):
                        nc.tensor.matmul(
                            op_ps[:],
                            o1_tiles[(h, j)][:, ti * 128:(ti + 1) * 128],
                            WoT[:, h * DIM + o: h * DIM + o + 512],
                            start=(h == 0), stop=False)
                    nc.tensor.matmul(op_ps[:], onesr[:, 0:128],
                                     bout[:, o:o + 512],
                                     start=False, stop=True)
                    ysb = ph4.tile([128, 512], F32, tag="ysb", name="ysb")
                    nc.vector.tensor_copy(ysb[:], op_ps[:])
                    nc.sync.dma_start(y[t * 128:(t + 1) * 128, o:o + 512],
                                      ysb[:])


_BUILT = {}


def _build(n_cores=N_CORES):
    if n_cores in _BUILT:
        return _BUILT[n_cores]
    nc = bacc.Bacc("TRN2", target_bir_lowering=False, debug=False,
                   num_devices=n_cores)
    in_specs = [
        ("xn_q", [T, DIM]), ("xn_k", [T, DIM]), ("xn_v", [T, DIM]),
        ("xT_q", [128, NT * DIM]), ("xT_k", [128, NT * DIM]),
        ("xT_v", [128, NT * DIM]),
        ("Wp", [128, 8 * DIM]), ("WoT", [128, 8 * DIM]),
        ("negu", [1, DIM]), ("vrow", [1, DIM]), ("bout", [1, DIM]),
        ("ones", [128, 128]), ("ident", [128, 128]), ("mask", [128, 1024]),
        ("wp1T", [128, 256]), ("wp2T", [128, 3]), ("b1row", [1, 128]),
        ("gbc", [8, 128]), ("bbc", [8, 128]), ("b2bc", [8, 3]),
    ]
    in_aps = [nc.dram_tensor(n, s, F32, kind="ExternalInput").ap()
              for n, s in in_specs]
    y_ap = nc.dram_tensor("y", [T, DIM], F32, kind="ExternalOutput").ap()
    with tile.TileContext(nc) as tc:
        attn_kernel(tc, [y_ap], in_aps, n_cores=n_cores)
    nc.compile()
    _BUILT[n_cores] = nc
    return nc


def kernel(q, k, v, ln_g, ln_b, w_in, wp_w1, wp_b1, wp_ln_g, wp_ln_b,
           wp_w2, wp_b2, w_out, b_out):
    q = np.asarray(q, dtype=np.float32)
    k = np.asarray(k, dtype=np.float32)
    v = np.asarray(v, dtype=np.float32)
    ln_g = np.asarray(ln_g, np.float32); ln_b = np.asarray(ln_b, np.float32)
    w_in = np.asarray(w_in, np.float32); w_out = np.asarray(w_out, np.float32)
    b_out = np.asarray(b_out, np.float32)
    wp_w1 = np.asarray(wp_w1, np.float32); wp_b1 = np.asarray(wp_b1, np.float32)
    wp_ln_g = np.asarray(wp_ln_g, np.float32)
    wp_ln_b = np.asarray(wp_ln_b, np.float32)
    wp_w2 = np.asarray(wp_w2, np.float32); wp_b2 = np.asarray(wp_b2, np.float32)

    # host weight prep (folded layernorm)
    W = w_in.T                                     # [DIM, HD]
    Wp = (ln_g[:, None] * W)
    negu = -(ln_g @ W)[None, :]
    vrow = (ln_b @ W)[None, :]
    Wp_t = np.ascontiguousarray(
        Wp.reshape(8, 128, 2, 512).transpose(1, 0, 2, 3)).reshape(128, -1)
    WoT = np.ascontiguousarray(
        w_out.T.reshape(8, 128, DIM).transpose(1, 0, 2)).reshape(128, -1)
    shared = {
        "Wp": Wp_t, "WoT": WoT, "negu": negu, "vrow": vrow,
        "bout": b_out[None, :],
        "ones": np.ones((128, 128), np.float32),
        "ident": np.eye(128, dtype=np.float32),
        "mask": np.tile((1.0 - np.eye(128)).astype(np.float32), (1, 8)),
        "wp1T": np.ascontiguousarray(wp_w1.T.reshape(2, 128, 128)
                                     .transpose(1, 0, 2)).reshape(128, 256),
        "wp2T": np.ascontiguousarray(wp_w2.T),
        "b1row": wp_b1[None, :],
        "gbc": np.tile(wp_ln_g[None, :], (8, 1)),
        "bbc": np.tile(wp_ln_b[None, :], (8, 1)),
        "b2bc": np.tile(wp_b2[None, :], (8, 1)),
    }
    shared = {kk: np.ascontiguousarray(vv, np.float32)
              for kk, vv in shared.items()}

    qf = q.reshape(QB * N, DIM)
    kf = k.reshape(QB * N, DIM)
    vf = v.reshape(QB * N, DIM)
    in_maps = []
    for c in range(N_CORES):
        sl = slice(c * T, (c + 1) * T)
        m = dict(shared)
        for nm, arr in (("q", qf[sl]), ("k", kf[sl]), ("v", vf[sl])):
            m[f"xn_{nm}"] = np.ascontiguousarray(arr)
            m[f"xT_{nm}"] = np.ascontiguousarray(
                arr.reshape(NT, 128, 8, 128).transpose(3, 0, 2, 1)
            ).reshape(128, NT * DIM)
        in_maps.append(m)

    nc = _build()
    res = bass_utils.run_bass_kernel_spmd(nc, in_maps,
                                          core_ids=list(range(N_CORES)))
    global LAST_RESULTS
    LAST_RESULTS = res
    out = np.concatenate([r["y"] for r in res.results], axis=0)
    return out.reshape(QB, N, DIM)


LAST_RESULTS = None



# revision 2
# speedup vs baseline: 1.5592x; 1.5592x over previous
"""Trainium2 Bass kernel v2 for nn_Attention_9096740733536 (sparse_attention).

Data-parallel over QB across 8 cores (2 tasks/core), two mid-kernel bf16
AllReduces (gk global sums early, feat_corr partials later).

vs v1: LayerNorm mean-subtraction is folded into the projection weight on the
host (W~ = gW - (1/D) 11^T gW), so phase 1 is pure matmuls with a per-token
1/sigma scale at PSUM eviction. All on-chip tensors are bf16 (f32r matmuls
with free-dim < 256 run at 1/4 rate; bf16 is full rate). The var-component
rank-1 term never materializes: mv comes out of free N=1 column matmuls,
q_ratio rows are partition-broadcast on the idle GPSIMD engine, and the
alpha/ww mixing is fused into the A-tile PSUM eviction (one DVE op per head).
Engines are strictly in-order, so blocks are emitted in intended schedule
order; the weight-predictor MLP and decorrelation chain are slotted between
PE phases so they overlap the collectives.
"""
import numpy as np
import ml_dtypes
from contextlib import ExitStack

import concourse.bass as bass
import concourse.tile as tile
from concourse import bacc, mybir
from concourse import bass_utils
from concourse._compat import with_exitstack

F32 = mybir.dt.float32
BF16 = mybir.dt.bfloat16
FP8 = mybir.dt.float8e4
AF = mybir.ActivationFunctionType
ALU = mybir.AluOpType
AX = mybir.AxisListType

H, D, DIM = 8, 128, 1024
QB, N = 16, 512
N_CORES = 8
T = QB * N // N_CORES          # 1024 tokens per core
NT = T // 128                  # 8 token tiles per core
NTASK = T // N                 # 2 tasks per core
LN_EPS = 1e-5
TOK_ALL = float(QB * N)
BF = np.dtype(ml_dtypes.bfloat16)
F8NP = np.dtype(ml_dtypes.float8_e4m3fn)


@with_exitstack
def attn_kernel(ctx: ExitStack, tc: tile.TileContext, outs, ins,
                n_cores=N_CORES, tok_all=TOK_ALL, has_ln_b=False,
                has_b_out=False):
    nc = tc.nc
    y = outs[0]
    (xn_q, xn_k, xn_v, xT_q, xT_k, xT_v, Wp_d, WoT_d, identb_d, identf_d,
     mask_d, wp1T_d, wp2T_d, b1_d, gbc_d, bbc_d, b2bc_d, vrow_d,
     bout_d) = ins

    consts = ctx.enter_context(tc.tile_pool(name="consts", bufs=1))
    fpool = ctx.enter_context(tc.tile_pool(name="fpool", bufs=1))
    statp = ctx.enter_context(tc.tile_pool(name="statp", bufs=1))
    dram = ctx.enter_context(tc.tile_pool(name="dram", bufs=1, space="DRAM"))

    ps_proj = ctx.enter_context(tc.tile_pool(name="ps_proj", bufs=3,
                                             space="PSUM"))
    ps_att = ctx.enter_context(tc.tile_pool(name="ps_att", bufs=3,
                                            space="PSUM"))
    ps_small = ps_att

    # ---- constants (allocated now, DMA'd later behind the input loads) ----
    identb = consts.tile([128, 128], BF16)
    identf = consts.tile([128, 128], F32)
    maskb = consts.tile([128, H * 128], BF16)
    wp1T = consts.tile([128, 256], F32)
    wp2T = consts.tile([128, 3], F32)
    b1row = consts.tile([1, 128], F32)
    gbc = consts.tile([8, 128], F32)
    bbc = consts.tile([8, 128], F32)
    b2bc = consts.tile([8, 3], F32)
    eps = consts.tile([128, 1], F32)
    nc.vector.memset(eps[:], LN_EPS)
    ones8 = consts.tile([1, 8], F32)
    nc.vector.memset(ones8[:], 1.0)
    onesb = consts.tile([128, 1], BF16)
    nc.vector.memset(onesb[:], 1.0)
    ones88 = consts.tile([128, 8], F32)
    nc.vector.memset(ones88[:], 1.0)
    if has_ln_b:
        vrow_sb = consts.tile([1, DIM], F32)
        nc.scalar.dma_start(vrow_sb[:], vrow_d[:])
        vrow_bc = consts.tile([128, DIM], F32)
        nc.gpsimd.partition_broadcast(vrow_bc[:], vrow_sb[:])
    if has_b_out:
        bout_bf = consts.tile([1, DIM], BF16)
        nc.scalar.dma_start(bout_bf[:], bout_d[:])
        onesrb = consts.tile([1, 128], BF16)
        nc.vector.memset(onesrb[:], 1.0)

    # ---- persistent F tensors (bf16): [128 tok, t*1024 + h*128 + d] ----
    Fq = fpool.tile([128, NT * DIM], BF16)
    Fk = fpool.tile([128, NT * DIM], BF16)
    Fv = fpool.tile([128, NT * DIM], BF16)
    Fs = [Fq, Fk, Fv]

    # per-(t,h) stats grids: col 2*(t*8+h) = mean, +1 = biased var
    qmv = statp.tile([128, 128], F32)
    kmv = statp.tile([128, 128], F32)
    qmun = statp.tile([128, 64], F32)   # negated mean (qc bias)
    qnin = statp.tile([128, 64], F32)   # 1/||Fq||
    qrt = statp.tile([128, 64], F32)    # q_ratio
    knin = statp.tile([128, 64], F32)   # 1/||Fk||
    krn = statp.tile([128, 64], BF16)   # k_ratio * ||Fk||  (bf16 for matmul)
    rscr = statp.tile([128, 128], F32)  # scratch
    rsig = statp.tile([128, 3 * NT], F32)  # LN 1/sigma, col t*3+i

    # pools that must not alias the phase-1 staging buffers (their work
    # overlaps the tail of phase 1)
    bn2 = ctx.enter_context(tc.tile_pool(name="bn2", bufs=4))
    qcpool = ctx.enter_context(tc.tile_pool(name="qcpool", bufs=64))

    def nr_rsqrt(dst, x, scratch, iters=2):
        """dst = 1/sqrt(x) for x = O(1), on the idle GPSIMD engine (ACT
        batches same-table activations together, starving evictions)."""
        nc.gpsimd.tensor_scalar(dst, x, -0.5, 1.5, ALU.mult, ALU.add)
        for _ in range(iters):
            nc.gpsimd.tensor_tensor(scratch, dst, dst, op=ALU.mult)
            nc.gpsimd.tensor_tensor(scratch, scratch, x, op=ALU.mult)
            nc.gpsimd.tensor_scalar(scratch, scratch, -0.5, 1.5,
                                    ALU.mult, ALU.add)
            nc.gpsimd.tensor_tensor(dst, dst, scratch, op=ALU.mult)

    # AR1: per-head global sums over Fq/Fk (only needs the q/k projections;
    # fired mid-phase-1 so the round trip hides under the v projection)
    ar1_in = dram.tile([128, 16], BF16)
    ar1_out = dram.tile([128, 16], BF16)
    ar2_in = dram.tile([128, H * 128], BF16)
    ar2_out = dram.tile([128, H * 128], BF16)
    gk_ps = ps_proj.tile([128, 16], F32, tag="gk", bufs=1)
    ph3 = ctx.enter_context(tc.tile_pool(name="ph3", bufs=2))
    arf1 = ph3.tile([128, 16], BF16, tag="arf1", bufs=1)
    featsq = statp.tile([128, 8], F32)
    featsk = statp.tile([128, 8], F32)

    def emit_ar1():
        for t in range(NT):
            for h in range(H):
                f_sl = slice(t * DIM + h * 128, t * DIM + h * 128 + 128)
                first = (h == 0 and t == 0)
                last = (h == H - 1 and t == NT - 1)
                nc.tensor.matmul(gk_ps[:, h:h + 1], Fq[:, f_sl], onesb[:],
                                 start=first, stop=last,
                                 skip_group_check=True)
                nc.tensor.matmul(gk_ps[:, 8 + h:9 + h], Fk[:, f_sl],
                                 onesb[:], start=False, stop=False,
                                 skip_group_check=True)
        gk_sb = ph3.tile([128, 16], BF16, tag="gksb", name="gk_sb")
        nc.scalar.copy(gk_sb[:], gk_ps[:])
        nc.sync.dma_start(ar1_in[:], gk_sb[:])
        if n_cores > 1:
            nc.gpsimd.collective_compute(
                "AllReduce", ALU.add,
                replica_groups=[list(range(n_cores))],
                ins=[ar1_in.opt()], outs=[ar1_out.opt()])
            arf1_src = ar1_out
        else:
            arf1_src = ar1_in
        nc.sync.dma_start(arf1[:], arf1_src[:])
        nc.vector.tensor_scalar_mul(featsq[:], arf1[:, 0:8], 1.0 / tok_all)
        nc.vector.tensor_scalar_mul(featsk[:], arf1[:, 8:16], 1.0 / tok_all)

    # ======== Phase 1: folded-LN projection ========
    with tc.tile_pool(name="xtp", bufs=3) as xtp, \
         tc.tile_pool(name="xnp", bufs=2) as xnp, \
         tc.tile_pool(name="bnp", bufs=4) as bnp:
        Wp = consts.tile([128, 8 * DIM], BF16)
        xns = [xn_q, xn_k, xn_v]
        xTs = [xT_q, xT_k, xT_v]
        xn_tiles = []
        xt_tiles = []
        for i in range(3):
            xn_tiles.append(xnp.tile([128, NT * DIM], FP8, tag="xn",
                                     name=f"xn{i}"))
            xt_tiles.append(xtp.tile([128, NT * DIM], BF16, tag="xt",
                                     name=f"xt{i}"))

        def ld_xn(i, q, quarter=None):
            src = xns[i].rearrange("(t p) d -> p t d", p=128)
            xn3 = xn_tiles[i][:, :].rearrange("p (t d) -> p t d", d=DIM)
            for ts_ in ([slice(0, 2), slice(2, 4)] if quarter == 0 else
                        [slice(4, 8)] if quarter == 1 else
                        [slice(0, 4), slice(4, 8)]):
                q.dma_start(xn3[:, ts_, :], src[:, ts_, :])

        def ld_xt(i, q, quarter=None):
            for ts_ in ([slice(0, 2), slice(2, 4)] if quarter == 0 else
                        [slice(4, 8)] if quarter == 1 else [slice(0, 8)]):
                o = ts_.start * DIM
                sz = (ts_.stop - ts_.start) * DIM
                q.dma_start(xt_tiles[i][:, o:o + sz], xTs[i][:, o:o + sz])

        # need-ordered across the two HWDGE queues (DMA engines are serial;
        # extra queues add no bandwidth, and SWDGE loads jump the queue)
        nc.sync.dma_start(Wp[:, 0:2048], Wp_d[:, 0:2048])
        ld_xt(0, nc.scalar, quarter=0)          # xT_q t01, t23
        nc.sync.dma_start(Wp[:, 2048:4096], Wp_d[:, 2048:4096])
        ld_xn(0, nc.sync, quarter=0)            # xn_q t01, t23
        nc.scalar.dma_start(Wp[:, 4096:6144], Wp_d[:, 4096:6144])
        nc.scalar.dma_start(Wp[:, 6144:8192], Wp_d[:, 6144:8192])
        ld_xn(0, nc.sync, quarter=1)            # xn_q t45-67
        ld_xt(0, nc.scalar, quarter=1)          # xT_q t45, t67
        ld_xt(1, nc.sync)
        ld_xn(1, nc.scalar)
        ld_xt(2, nc.sync)
        ld_xn(2, nc.sync)

        def bn_block(i, trange):
            xn_b = xn_tiles[i]
            for t in trange:
                bnt = bnp.tile([128, 12], F32, tag="bnt")
                nc.vector.bn_stats(bnt[:, 0:6],
                                   xn_b[:, t * DIM:t * DIM + 512])
                nc.vector.bn_stats(bnt[:, 6:12],
                                   xn_b[:, t * DIM + 512:(t + 1) * DIM])
                nc.vector.bn_aggr(rscr[:, 2 * t:2 * t + 2], bnt[:])
            t0, t1 = trange[0], trange[-1]
            w = len(trange)
            sg = bnp.tile([128, 8], F32, tag="sg")
            nc.gpsimd.tensor_scalar_add(sg[:, 0:w],
                                        rscr[:, 2 * t0 + 1:2 * t1 + 2:2],
                                        LN_EPS)
            nr_rsqrt(rsig[:, t0 * 3 + i:t1 * 3 + i + 1:3], sg[:, 0:w],
                     sg[:, 4:4 + w])

        def proj_block(i, half, trange):
            xt_b = xt_tiles[i]
            o = half * 512
            for t in trange:
                rcol = rsig[:, t * 3 + i:t * 3 + i + 1]
                acc = ps_proj.tile([128, 512], F32, tag="proj")
                for s_ in range(8):
                    nc.tensor.matmul(
                        acc[:],
                        xt_b[:, t * DIM + s_ * 128:
                             t * DIM + (s_ + 1) * 128],
                        Wp[:, o * 8 + s_ * 512:o * 8 + (s_ + 1) * 512],
                        start=(s_ == 0), stop=(s_ == 7))
                dst = Fs[i][:, t * DIM + o:t * DIM + o + 512]
                if has_ln_b:
                    nc.vector.scalar_tensor_tensor(
                        dst, acc[:], rcol, vrow_bc[:, o:o + 512],
                        op0=ALU.mult, op1=ALU.add)
                else:
                    nc.scalar.mul(dst, acc[:], rcol)

        def do_tensor(i):
            bn_block(i, range(0, 2))
            bn_block(i, range(2, 4))
            proj_block(i, 0, range(0, 4))
            proj_block(i, 1, range(0, 4))
            bn_block(i, range(4, 6))
            bn_block(i, range(6, 8))
            proj_block(i, 0, range(4, 8))
            proj_block(i, 1, range(4, 8))

        do_tensor(0)
        do_tensor(1)
        emit_ar1()
        do_tensor(2)

    late = ctx.enter_context(tc.tile_pool(name="late", bufs=1))

    # late-needed constants (queued behind the phase-1 input loads)
    nc.sync.dma_start(identb[:], identb_d[:])
    nc.scalar.dma_start(identf[:], identf_d[:])
    nc.sync.dma_start(maskb[:], mask_d[:])
    nc.scalar.dma_start(wp1T[:], wp1T_d[:])
    nc.sync.dma_start(wp2T[:], wp2T_d[:])
    nc.scalar.dma_start(b1row[:], b1_d[:])
    nc.sync.dma_start(gbc[:], gbc_d[:])
    nc.scalar.dma_start(bbc[:], bbc_d[:])
    nc.sync.dma_start(b2bc[:], b2bc_d[:])

    # ======== Phase 2: per-(t,h) stats on F ========
    for F, grid in ((Fq, qmv), (Fk, kmv)):
        for t in range(NT):
            b48 = bn2.tile([128, 48], F32, tag="b48")
            for h in range(H):
                c = t * 8 + h
                nc.vector.bn_stats(
                    b48[:, 6 * h:6 * h + 6],
                    F[:, t * DIM + h * 128:t * DIM + (h + 1) * 128])
                nc.vector.bn_aggr(grid[:, 2 * c:2 * c + 2],
                                  b48[:, 6 * h:6 * h + 6])

    def derived(grid, jh, ratio, ninv, n_mean=None):
        # cols for t in [4jh, 4jh+4): mean = grid[:, 64jh::2], var likewise
        base = 64 * jh
        m = grid[:, base:base + 64:2]
        v = grid[:, base + 1:base + 64:2]
        w = 32
        t1 = rscr[:, 0:w]
        t2 = rscr[:, w:2 * w]
        t3 = rscr[:, 2 * w:3 * w]
        # s = var + mean^2 = sumsq/D (O(1)); y = rsqrt(s) via GPSIMD NR
        # ninv = y/sqrt(D); norm = sqrt(sumsq) = s*y*sqrt(D)
        t4 = rscr[:, 96:128] if n_mean is not None else rscr[:, 64:96]
        nc.vector.tensor_tensor(t1, m, m, op=ALU.mult)
        nc.vector.tensor_tensor(t1, v, t1, op=ALU.add)
        nr_rsqrt(t2, t1, t4, iters=3)
        nc.vector.tensor_scalar_mul(ninv, t2, float(D) ** -0.5)
        nc.vector.tensor_tensor(t2, t1, t2, op=ALU.mult)
        nc.vector.tensor_scalar_mul(t2, t2, float(D) ** 0.5)
        # unbiased var = var * D/(D-1); ratio = 2*min(vu,1)/(vu+1)
        nc.vector.tensor_scalar_mul(t3, v, float(D) / (D - 1))
        nc.vector.tensor_scalar(t1, t3, 1.0, 2.0, ALU.min, ALU.mult)
        nc.vector.tensor_scalar_add(t3, t3, 1.0)
        nc.vector.reciprocal(t3, t3)
        nc.vector.tensor_tensor(ratio, t1, t3, op=ALU.mult)
        if n_mean is not None:
            nc.vector.tensor_scalar_mul(n_mean, m, -1.0)
        return t2  # t2 holds sqrt(sumsq) = norm (scratch!)

    for jh in range(NTASK):
        sl = slice(jh * 32, jh * 32 + 32)
        derived(qmv, jh, qrt[:, sl], qnin[:, sl], n_mean=qmun[:, sl])
        krt_dummy = rscr[:, 96:128]
        kn = derived(kmv, jh, krt_dummy, knin[:, sl])
        # krn = k_ratio * kn (bf16)
        nc.vector.tensor_tensor(krn[:, sl], krt_dummy, kn, op=ALU.mult)
        # scale Fv in place by 1/kn
        for t in range(4 * jh, 4 * jh + 4):
            for h in range(H):
                f_sl = slice(t * DIM + h * 128, t * DIM + h * 128 + 128)
                nc.vector.tensor_scalar(
                    Fv[:, f_sl], Fv[:, f_sl],
                    knin[:, t * 8 + h:t * 8 + h + 1], None, ALU.mult)

    # ---- qc (+ in-place Fq/qn scaling right behind it) ----
    qc_tiles = {}
    for t in range(NT):
        for h in range(H):
            f_sl = slice(t * DIM + h * 128, t * DIM + h * 128 + 128)
            c = t * 8 + h
            qc = qcpool.tile([128, 128], BF16, tag="qc", name="qc")
            nc.vector.tensor_scalar(qc[:], Fq[:, f_sl],
                                    qmun[:, c:c + 1], None, ALU.add)
            qc_tiles[(t, h)] = qc
            nc.vector.tensor_scalar(Fq[:, f_sl], Fq[:, f_sl],
                                    qnin[:, c:c + 1], None, ALU.mult)

    # ---- weight-predictor MLP part 1 (AR1 data ready during phase 1) ----
    ps_sm = ps_att
    h1_ps = ps_sm.tile([8, 128], F32, tag="sm", name="h1_ps", bufs=1)
    nc.tensor.matmul(h1_ps[:], featsq[:], wp1T[:, 0:128], start=True,
                     stop=False)
    nc.tensor.matmul(h1_ps[:], featsk[:], wp1T[:, 128:256], start=False,
                     stop=False)
    nc.tensor.matmul(h1_ps[:], ones8[:], b1row[:], start=False, stop=True)
    h1 = statp.tile([8, 128], F32)
    nc.scalar.copy(h1[:], h1_ps[:])
    wbn = statp.tile([8, 8], F32)
    nc.vector.bn_stats(wbn[:, 0:6], h1[:])
    nc.vector.bn_aggr(wbn[:, 6:8], wbn[:, 0:6])
    w_mu = statp.tile([8, 2], F32)
    nc.scalar.activation(w_mu[:, 0:1], wbn[:, 7:8], AF.Sqrt, bias=eps[0:8, :])
    nc.vector.reciprocal(w_mu[:, 0:1], w_mu[:, 0:1])
    h1n = statp.tile([8, 128], F32)
    nc.vector.tensor_scalar(h1n[:], h1[:], wbn[:, 6:7], w_mu[:, 0:1],
                            ALU.subtract, ALU.mult)
    nc.vector.tensor_tensor(h1n[:], h1n[:], gbc[:], op=ALU.mult)
    nc.vector.tensor_tensor(h1n[:], h1n[:], bbc[:], op=ALU.add)
    nc.vector.tensor_scalar_max(h1n[:], h1n[:], 0.0)

    # ---- feat_corr -> AR2 ----
    for hh in range(2):  # 4-head groups
        fc_ps = ps_att.tile([128, 512], F32, tag="att512", name="fc_ps")
        for hi in range(4):
            h = hh * 4 + hi
            for t in range(NT):
                nc.tensor.matmul(fc_ps[:, hi * 128:(hi + 1) * 128],
                                 qc_tiles[(t, h)][:], qc_tiles[(t, h)][:],
                                 start=(t == 0), stop=(t == NT - 1),
                                 skip_group_check=True)
        fc_sb = ph3.tile([128, 512], BF16, tag="fcsb", name="fc_sb")
        nc.scalar.copy(fc_sb[:], fc_ps[:])
        nc.sync.dma_start(ar2_in[:, hh * 512:(hh + 1) * 512], fc_sb[:])

    if n_cores > 1:
        nc.gpsimd.collective_compute(
            "AllReduce", ALU.add,
            replica_groups=[list(range(n_cores))],
            ins=[ar2_in.opt()], outs=[ar2_out.opt()])
        arf2_src = ar2_out
    else:
        arf2_src = ar2_in
    arf2 = late.tile([128, H * 128], BF16)
    nc.sync.dma_start(arf2[:], arf2_src[:])

    # output-projection weights (needed only at phase 5)
    WoT = late.tile([128, 8 * DIM], BF16)
    nc.gpsimd.dma_start(WoT[:, 0:4096], WoT_d[:, 0:4096])
    nc.gpsimd.dma_start(WoT[:, 4096:8192], WoT_d[:, 4096:8192])

    # ---- early-parked chain heads (wait on AR legs in the wait queue) ----
    msk = late.tile([128, H * 128], BF16)
    nc.vector.tensor_tensor(msk[:], arf2[:], maskb[:], op=ALU.mult)
    sq_scr = late.tile([128, H * 128], BF16)
    nc.scalar.activation(sq_scr[:], msk[:], AF.Square, scale=1.0 / tok_all)

    # ---- M and mv columns for both tasks (AR-independent PE work) ----
    attn = ctx.enter_context(tc.tile_pool(name="attn", bufs=1))
    mmp = ctx.enter_context(tc.tile_pool(name="mmp", bufs=4))
    fqtp = ctx.enter_context(tc.tile_pool(name="fqtp", bufs=16))
    mm_sbs = {}
    mvw = {}
    for j in range(NTASK):
        mvc_ps = ps_proj.tile([128, 8], F32, tag="gk", name="mvc_ps",
                              bufs=1)
        for hh in range(2):
            mm_ps = ps_att.tile([128, 512], F32, tag="att512", name="mm_ps")
            for hi in range(4):
                h = hh * 4 + hi
                for ti in range(4):
                    t = 4 * j + ti
                    f_sl = slice(t * DIM + h * 128, t * DIM + h * 128 + 128)
                    nc.tensor.matmul(mm_ps[:, hi * 128:(hi + 1) * 128],
                                     Fk[:, f_sl], Fv[:, f_sl],
                                     start=(ti == 0), stop=(ti == 3),
                                     skip_group_check=True)
                    nc.tensor.matmul(
                        mvc_ps[:, h:h + 1], Fv[:, f_sl],
                        krn[:, (t * 8 + h):(t * 8 + h) + 1],
                        start=(ti == 0), stop=(ti == 3),
                        skip_group_check=True)
            mm_sb = mmp.tile([128, 512], BF16, tag="mmsb", name="mm_sb")
            if hh == 0:
                nc.scalar.copy(mm_sb[:], mm_ps[:])
            else:
                nc.vector.tensor_copy(mm_sb[:], mm_ps[:])
            mm_sbs[(j, hh)] = mm_sb
        mv_j = statp.tile([128, 8], F32, name=f"mv{j}")
        nc.vector.tensor_copy(mv_j[:], mvc_ps[:])
        mvw[j] = mv_j

    ssq = statp.tile([128, 8], F32)
    nc.vector.reduce_sum(ssq[:],
                         sq_scr[:].rearrange("p (h d) -> p h d", h=8),
                         axis=AX.X)
    h1T_ps = ps_proj.tile([128, 8], F32, tag="gk", name="h1T_ps",
                          bufs=1)
    nc.tensor.transpose(h1T_ps[:], h1n[:], identf[0:8, 0:8])
    h1T = statp.tile([128, 8], F32)
    nc.scalar.copy(h1T[:], h1T_ps[:])
    lg_ps = ps_sm.tile([8, 3], F32, tag="sm", name="lg_ps", bufs=1)
    nc.tensor.matmul(lg_ps[:], h1T[:], wp2T[:], start=True, stop=True)
    lg = statp.tile([8, 8], F32)
    nc.scalar.copy(lg[:, 0:3], lg_ps[:])
    nc.vector.tensor_tensor(lg[:, 0:3], lg[:, 0:3], b2bc[:], op=ALU.add)
    # dsc sqrt BEFORE the exps: keeps ACT in the sqrt table until here,
    # then a single switch to the exp table for the rest of the kernel
    ss_ps = ps_proj.tile([8, 8], F32, tag="gk", name="ss_ps", bufs=1)
    nc.tensor.matmul(ss_ps[:], ssq[:], ones88[:], start=True, stop=True)
    dsc = statp.tile([8, 8], F32)
    nc.scalar.activation(dsc[:, 0:1], ss_ps[0:8, 0:1], AF.Sqrt)
    nc.scalar.activation(lg[:, 0:3], lg[:, 0:3], AF.Exp)
    nc.scalar.activation(dsc[:, 1:2], dsc[:, 0:1], AF.Exp,
                         scale=-5.0 / (D * D))
    nc.vector.reduce_sum(lg[:, 4:5], lg[:, 0:3], axis=AX.X)
    nc.vector.reciprocal(lg[:, 4:5], lg[:, 4:5])
    nc.vector.tensor_scalar(lg[:, 0:3], lg[:, 0:3], lg[:, 4:5], None,
                            ALU.mult)

    # ---- q_ratio rows (h,t)-ordered -> [1,8192] -> GPSIMD broadcasts ----
    qrh = statp.tile([128, 64], F32)
    nc.vector.tensor_copy(qrh[:],
                          qrt[:, :].rearrange("p (t h) -> p h t", h=8))
    qrT_ps = ps_sm.tile([64, 128], F32, tag="sm", name="qrT_ps", bufs=1)
    nc.tensor.matmul(qrT_ps[:], qrh[:], identf[:], is_transpose=True,
                     start=True, stop=True, skip_group_check=True)
    qrT = late.tile([64, 128], BF16)
    nc.scalar.copy(qrT[:], qrT_ps[:])
    wqr = late.tile([1, 8192], BF16)
    nc.sync.dma_start(wqr[:], qrT[:])
    qrbc_tiles = {}
    for j in range(NTASK):
        for h in range(H):
            qrbc = attn.tile([128, 512], BF16, tag=f"qr{h}{j}", name="qrbc")
            nc.gpsimd.partition_broadcast(
                qrbc[:], wqr[0:1, h * 1024 + j * 512:h * 1024 + j * 512 + 512])
            qrbc_tiles[(h, j)] = qrbc

    # ---- ww broadcast (independent of dsc) + mv*ww ----
    awT2_ps = ps_sm.tile([1, 8], F32, tag="sm", name="awT2_ps", bufs=1)
    nc.tensor.matmul(awT2_ps[:], lg[:, 2:3], identf[0:8, 0:8],
                     is_transpose=True, start=True, stop=True,
                     skip_group_check=True)
    awT2 = statp.tile([1, 8], F32)
    nc.scalar.copy(awT2[:], awT2_ps[:])
    wbc = statp.tile([128, 8], F32)
    nc.gpsimd.partition_broadcast(wbc[:], awT2[:])
    for j in range(NTASK):
        nc.vector.tensor_tensor(mvw[j][:], mvw[j][:], wbc[:], op=ALU.mult)

    # ---- alpha broadcast ----
    aw = statp.tile([8, 2], F32)
    nc.vector.tensor_tensor(aw[:, 0:1], lg[:, 1:2], dsc[:, 1:2], op=ALU.mult)
    nc.vector.tensor_tensor(aw[:, 0:1], aw[:, 0:1], lg[:, 0:1], op=ALU.add)
    awT_ps = ps_proj.tile([1, 8], F32, tag="gk", name="awT_ps",
                          bufs=1)
    nc.tensor.transpose(awT_ps[:], aw[:, 0:1], identf[0:8, 0:8])
    awT = statp.tile([1, 8], F32)
    nc.scalar.copy(awT[:], awT_ps[:])
    abc = statp.tile([128, 8], F32)
    nc.gpsimd.partition_broadcast(abc[:], awT[:])

    # ======== Phase 4/5: alpha-scaled transposes, A+fused o1, out-proj ====
    fqT_tiles = {}

    def qsT_sweep(j):
        # fqT = alpha_h * Fq-hat^T : the alpha scale rides the PSUM eviction
        for h in range(H):
            qsT_ps = ps_att.tile([128, 512], BF16, tag="att512",
                                 name="qsT_ps")
            for ti in range(4):
                t = 4 * j + ti
                f_sl = slice(t * DIM + h * 128, t * DIM + h * 128 + 128)
                nc.tensor.matmul(qsT_ps[:, ti * 128:(ti + 1) * 128],
                                 Fq[:, f_sl], identb[:],
                                 is_transpose=True, start=True,
                                 stop=True, skip_group_check=True)
            fqT = fqtp.tile([128, 512], BF16, tag="fqT", name="fqT")
            if h % 2 == 0:
                nc.scalar.mul(fqT[:], qsT_ps[:], abc[:, h:h + 1])
            else:
                nc.vector.tensor_scalar(fqT[:], qsT_ps[:], abc[:, h:h + 1],
                                        None, ALU.mult)
            fqT_tiles[(h, j)] = fqT

    o1_tiles = {}

    def a_sweep(j):
        for h in range(H):
            hh, hi = divmod(h, 4)
            # A = M^T @ (alpha Fq^T)   [e, tok]
            A_ps = ps_att.tile([128, 512], F32, tag="att512", name="A_ps")
            nc.tensor.matmul(A_ps[:],
                             mm_sbs[(j, hh)][:, hi * 128:(hi + 1) * 128],
                             fqT_tiles[(h, j)][:], start=True, stop=True)
            # fused eviction: o1 = (q_ratio * ww*mv) + A
            o1 = attn.tile([128, 512], BF16, tag=f"A{h}{j}", name="o1")
            nc.vector.scalar_tensor_tensor(
                o1[:], qrbc_tiles[(h, j)][:], mvw[j][:, h:h + 1], A_ps[:],
                op0=ALU.mult, op1=ALU.add)
            o1_tiles[(h, j)] = o1

    def op_proj(j, ysp):
        for ti in range(4):
            t = 4 * j + ti
            for half in range(2):
                o = half * 512
                op_ps = ps_proj.tile([128, 512], F32, tag="proj",
                                     name="op_ps")
                for h in range(H):
                    nc.tensor.matmul(
                        op_ps[:],
                        o1_tiles[(h, j)][:, ti * 128:(ti + 1) * 128],
                        WoT[:, h * DIM + o:h * DIM + o + 512],
                        start=(h == 0),
                        stop=(h == 7 and not has_b_out))
                if has_b_out:
                    nc.tensor.matmul(op_ps[:], onesrb[:],
                                     bout_bf[:, o:o + 512],
                                     start=False, stop=True)
                ysb = ysp.tile([128, 512], F32, tag="ysb", name="ysb")
                if half == 0:
                    nc.scalar.copy(ysb[:], op_ps[:])
                else:
                    nc.vector.tensor_copy(ysb[:], op_ps[:])
                nc.sync.dma_start(y[t * 128:(t + 1) * 128, o:o + 512],
                                  ysb[:])

    with tc.tile_pool(name="ysp", bufs=3) as ysp:
        qsT_sweep(0)
        a_sweep(0)
        qsT_sweep(1)
        op_proj(0, ysp)
        a_sweep(1)
        op_proj(1, ysp)


_BUILT = {}



def _build(n_cores=N_CORES, tok_all=TOK_ALL, has_ln_b=False, has_b_out=False):
    key = (n_cores, tok_all, has_ln_b, has_b_out)
    if key in _BUILT:
        return _BUILT[key]
    nc = bacc.Bacc("TRN2", target_bir_lowering=False, debug=False,
                   num_devices=n_cores)
    in_specs = [
        ("xn_q", [T, DIM], FP8), ("xn_k", [T, DIM], FP8),
        ("xn_v", [T, DIM], FP8),
        ("xT_q", [128, NT * DIM], BF16), ("xT_k", [128, NT * DIM], BF16),
        ("xT_v", [128, NT * DIM], BF16),
        ("Wp", [128, 8 * DIM], BF16), ("WoT", [128, 8 * DIM], BF16),
        ("identb", [128, 128], BF16), ("identf", [128, 128], F32),
        ("mask", [128, 1024], BF16),
        ("wp1T", [128, 256], F32), ("wp2T", [128, 3], F32),
        ("b1row", [1, 128], F32),
        ("gbc", [8, 128], F32), ("bbc", [8, 128], F32), ("b2bc", [8, 3], F32),
        ("vrow", [1, DIM], F32), ("bout", [1, DIM], BF16),
    ]
    in_aps = [nc.dram_tensor(n, s, d, kind="ExternalInput").ap()
              for n, s, d in in_specs]
    y_ap = nc.dram_tensor("y", [T, DIM], F32, kind="ExternalOutput").ap()
    with tile.TileContext(nc) as tc:
        attn_kernel(tc, [y_ap], in_aps, n_cores=n_cores, tok_all=tok_all,
                    has_ln_b=has_ln_b, has_b_out=has_b_out)
    nc.compile()
    _BUILT[key] = nc
    return nc


def prepare_maps(q, k, v, ln_g, ln_b, w_in, wp_w1, wp_b1, wp_ln_g, wp_ln_b,
                 wp_w2, wp_b2, w_out, b_out):
    """Host-side prep: fold LN gain+mean into W, bf16 casts, shard."""
    q = np.asarray(q, dtype=np.float32)
    k = np.asarray(k, dtype=np.float32)
    v = np.asarray(v, dtype=np.float32)
    ln_g = np.asarray(ln_g, np.float32)
    ln_b = np.asarray(ln_b, np.float32)
    w_in = np.asarray(w_in, np.float32)
    w_out = np.asarray(w_out, np.float32)
    b_out = np.asarray(b_out, np.float32)
    wp_w1 = np.asarray(wp_w1, np.float32)
    wp_b1 = np.asarray(wp_b1, np.float32)
    wp_ln_g = np.asarray(wp_ln_g, np.float32)
    wp_ln_b = np.asarray(wp_ln_b, np.float32)
    wp_w2 = np.asarray(wp_w2, np.float32)
    wp_b2 = np.asarray(wp_b2, np.float32)

    Wg = ln_g[:, None] * w_in.T                     # [DIM, HD]
    Wt = Wg - np.sum(Wg, axis=0, keepdims=True) / DIM   # fold mean-subtract
    vrow = (ln_b @ Wg)[None, :]
    # half-major layout: [c, half, s, o]
    Wp_t = np.ascontiguousarray(
        Wt.reshape(8, 128, 2, 512).transpose(1, 2, 0, 3)).reshape(128, -1)
    WoT = np.ascontiguousarray(
        w_out.T.reshape(8, 128, DIM).transpose(1, 0, 2)).reshape(128, -1)
    shared = {
        "Wp": Wp_t.astype(BF), "WoT": WoT.astype(BF),
        "identb": np.eye(128).astype(BF),
        "identf": np.eye(128, dtype=np.float32),
        "mask": np.tile((1.0 - np.eye(128)), (1, 8)).astype(BF),
        "wp1T": np.ascontiguousarray(
            wp_w1.T.reshape(2, 128, 128).transpose(1, 0, 2))
        .reshape(128, 256).astype(np.float32),
        "wp2T": np.ascontiguousarray(wp_w2.T).astype(np.float32),
        "b1row": wp_b1[None, :].astype(np.float32),
        "gbc": np.tile(wp_ln_g[None, :], (8, 1)).astype(np.float32),
        "bbc": np.tile(wp_ln_b[None, :], (8, 1)).astype(np.float32),
        "b2bc": np.tile(wp_b2[None, :], (8, 1)).astype(np.float32),
        "vrow": vrow.astype(np.float32),
        "bout": b_out[None, :].astype(BF),
    }
    shared = {kk: np.ascontiguousarray(vv) for kk, vv in shared.items()}

    qf = q.reshape(QB * N, DIM)
    kf = k.reshape(QB * N, DIM)
    vf = v.reshape(QB * N, DIM)
    in_maps = []
    for c in range(N_CORES):
        sl = slice(c * T, (c + 1) * T)
        m = dict(shared)
        for nm, arr in (("q", qf[sl]), ("k", kf[sl]), ("v", vf[sl])):
            ab = arr.astype(BF)
            m[f"xn_{nm}"] = np.ascontiguousarray(arr.astype(F8NP))
            m[f"xT_{nm}"] = np.ascontiguousarray(
                ab.reshape(NT, 128, 8, 128).transpose(3, 0, 2, 1)
            ).reshape(128, NT * DIM)
        in_maps.append(m)
    has_ln_b = bool(np.any(ln_b != 0.0))
    has_b_out = bool(np.any(b_out != 0.0))
    return in_maps, has_ln_b, has_b_out


def kernel(q, k, v, ln_g, ln_b, w_in, wp_w1, wp_b1, wp_ln_g, wp_ln_b,
           wp_w2, wp_b2, w_out, b_out):
    in_maps, has_ln_b, has_b_out = prepare_maps(
        q, k, v, ln_g, ln_b, w_in, wp_w1, wp_b1, wp_ln_g, wp_ln_b,
        wp_w2, wp_b2, w_out, b_out)
    nc = _build(has_ln_b=has_ln_b, has_b_out=has_b_out)
    res = bass_utils.run_bass_kernel_spmd(nc, in_maps,
                                          core_ids=list(range(N_CORES)))
    global LAST_RESULTS
    LAST_RESULTS = res
    out = np.concatenate([r["y"] for r in res.results], axis=0)
    return out.reshape(QB, N, DIM)


LAST_RESULTS = None
